# revision 8
# baseline (speedup 1.0000x reference)
"""Causal depthwise conv (kernel_size=4) on 8 TRN2 NeuronCores.

Problem: x (4, 4096, 16, 128) f32, weight (4, 16, 128) f32,
out[b,t,h,d] = sum_k weight[k,h,d] * x[b,t-k,h,d]   (zero-pad t<0).

Sharding: tensor-parallel over heads - core c owns heads [2c, 2c+2).
Host transposes each core's slice to d-major layout: on-device partition
dim is d (128), free dim is t; stream j = hl*BATCH + b.  The graded
rel-err threshold (2e-2) leaves ~25x margin for fp16 I/O, which halves
the HBM traffic vs f32 (16.8 MB/core at ~330-360 GB/s -> ~50 us floor).

In fp16 a pure DVE tap loop becomes the bottleneck (scalar_tensor_tensor
has no packed DVE mode -> ~4.3 us per stream-tap), so taps are spread
over four engines:

  head 0 (streams 0-3): TensorE.  conv = 4 PSUM-accumulated matmuls with
    stationary diag(weight[k,head,:]) and shifted moving slices of x;
    ScalarE evacuates PSUM -> fp16 SBUF (one 2048-col activation per
    half-stream).
  head 1 (streams 4-7): ScalarE k=0 (activation copy with per-partition
    scale); VectorE k=1,3 via scalar_tensor_tensor (1x mode - STT has no
    packed uops); k=2 as a 4x-mode tensor_scalar multiply (4B-aligned
    even-shift read) whose add lands on GpSimd tensor_tensor (the Pool
    engine legally runs TT but not STT), keeping every engine under the
    DMA floor.

Input DMAs issue from the sync (SP) HWDGE ring, output DMAs from the
scalar (Activation) ring so neither FIFO head-of-line blocks the other.
Every DMA row is one contiguous ~8.2 KB run; per-stream DMAs are ~1 MB.
"""

import time

import numpy as np

import concourse.mybir as mybir
from concourse import bacc, tile
from concourse.bass_utils import run_bass_kernel_spmd

BATCH, SEQ, N_HEADS, D_HEAD = 4, 4096, 16, 128
KERNEL = 4
PAD = 4                                  # leading zero columns per stream
N_CORES = 8
H_PER_CORE = N_HEADS // N_CORES          # 2
N_STREAMS = H_PER_CORE * BATCH           # 8 per core; stream j = hl*BATCH + b
PE_HEAD = 0                              # local head handled by TensorE

F16 = mybir.dt.float16
F32 = mybir.dt.float32

CHUNK = 2048                             # PSUM tile cols (4 banks)
MM = 512                                 # matmul moving free-dim limit

last_results = None


def _build_module(repeats: int = 1, seq: int = SEQ, mode: str = "full"):
    nc = bacc.Bacc(
        "TRN2",
        target_bir_lowering=False,
        debug=False,
        num_devices=N_CORES,
        enable_asserts=False,
    )
    x = nc.dram_tensor("x", [D_HEAD, N_STREAMS, seq + PAD], F16, kind="ExternalInput").ap()
    w = nc.dram_tensor("w", [D_HEAD, H_PER_CORE * KERNEL], F32, kind="ExternalInput").ap()
    wd = nc.dram_tensor("wd", [D_HEAD, KERNEL, D_HEAD], F16, kind="ExternalInput").ap()
    out = nc.dram_tensor("out", [D_HEAD, N_STREAMS, seq], F16, kind="ExternalOutput").ap()
    n_chunks = seq // CHUNK
    assert seq % CHUNK == 0

    with tile.TileContext(nc) as tc:
        with (
            tc.tile_pool(name="wp", bufs=1) as wp,
            tc.tile_pool(name="xp", bufs=8) as xp,
            tc.tile_pool(name="op", bufs=8) as op,
            tc.tile_pool(name="tp", bufs=4) as tp,
            tc.tile_pool(name="pp", bufs=2, space="PSUM") as pp,
        ):
            wt = wp.tile([D_HEAD, H_PER_CORE * KERNEL], F32)
            nc.sync.dma_start(out=wt, in_=w)
            wdt = wp.tile([D_HEAD, KERNEL, D_HEAD], F16)
            nc.sync.dma_start(out=wdt, in_=wd)
            if mode == "dma":
                OZ = wp.tile([D_HEAD, seq], F16)
                nc.vector.memset(OZ, 0.0)
                for _r in range(repeats):
                    for j in range(N_STREAMS):
                        X = xp.tile([D_HEAD, seq + PAD], F16, tag="x")
                        nc.sync.dma_start(out=X, in_=x[:, j, :])
                        nc.scalar.dma_start(out=out[:, j, :], in_=OZ)
                nc.compile()
                return nc
            if mode == "nodma":
                XZ = wp.tile([D_HEAD, seq + PAD], F16)
                nc.vector.memset(XZ, 0.0)
            for _r in range(repeats):
                for j in range(N_STREAMS):
                    hl = j // BATCH
                    if mode == "nodma":
                        X = XZ
                    else:
                        X = xp.tile([D_HEAD, seq + PAD], F16, tag="x")
                        nc.sync.dma_start(out=X, in_=x[:, j, :])
                    O = op.tile([D_HEAD, seq], F16, tag="o")
                    if hl == PE_HEAD:
                        for c in range(n_chunks):
                            pt = pp.tile([D_HEAD, CHUNK], F32, tag="ps")
                            for m in range(CHUNK // MM):
                                col = c * CHUNK + m * MM
                                for k in range(KERNEL):
                                    nc.tensor.matmul(
                                        pt[:, m * MM : (m + 1) * MM],
                                        lhsT=wdt[:, k, :],
                                        rhs=X[:, PAD + col - k : PAD + col - k + MM],
                                        start=(k == 0),
                                        stop=(k == KERNEL - 1),
                                    )
                            nc.scalar.activation(
                                O[:, c * CHUNK : (c + 1) * CHUNK], pt,
                                mybir.ActivationFunctionType.Copy,
                            )
                    else:
                        def wcol(k):
                            return wt[:, hl * KERNEL + k : hl * KERNEL + k + 1]

                        T2 = tp.tile([D_HEAD, seq], F16, tag="t2")
                        nc.vector.tensor_scalar_mul(
                            T2, X[:, PAD - 2 : PAD - 2 + seq], wcol(2)
                        )
                        nc.scalar.activation(
                            O, X[:, PAD : PAD + seq],
                            mybir.ActivationFunctionType.Copy, scale=wcol(0),
                        )
                        nc.vector.scalar_tensor_tensor(
                            O, X[:, PAD - 1 : PAD - 1 + seq], wcol(1), O,
                            mybir.AluOpType.mult, mybir.AluOpType.add,
                        )
                        nc.gpsimd.tensor_tensor(
                            O, O, T2, mybir.AluOpType.add
                        )
                        nc.vector.scalar_tensor_tensor(
                            O, X[:, PAD - 3 : PAD - 3 + seq], wcol(3), O,
                            mybir.AluOpType.mult, mybir.AluOpType.add,
                        )
                    if mode != "nodma":
                        nc.scalar.dma_start(out=out[:, j, :], in_=O)
    nc.compile()
    return nc


_module = None


def _get_module():
    global _module
    if _module is None:
        _module = _build_module()
    return _module


def _shard_inputs(x: np.ndarray, weight: np.ndarray, seq: int = SEQ):
    in_maps = []
    for c in range(N_CORES):
        h0 = c * H_PER_CORE
        xs = x[:, :, h0 : h0 + H_PER_CORE, :]                # (B, T, HL, D)
        xt = xs.transpose(3, 2, 0, 1)                        # (D, HL, B, T)
        xin = np.zeros((D_HEAD, N_STREAMS, seq + PAD), dtype=np.float16)
        xin[:, :, PAD:] = xt.reshape(D_HEAD, N_STREAMS, seq).astype(np.float16)
        ws = weight[:, h0 : h0 + H_PER_CORE, :]              # (K, HL, D)
        warr = np.ascontiguousarray(ws.transpose(2, 1, 0)).reshape(
            D_HEAD, H_PER_CORE * KERNEL
        ).astype(np.float32)
        wdh = np.zeros((D_HEAD, KERNEL, D_HEAD), dtype=np.float16)
        idx = np.arange(D_HEAD)
        for k in range(KERNEL):
            wdh[idx, k, idx] = weight[k, h0 + PE_HEAD, :].astype(np.float16)
        in_maps.append({"x": xin, "w": warr, "wd": wdh})
    return in_maps


def _unshard(results, seq: int = SEQ) -> np.ndarray:
    out = np.empty((BATCH, seq, N_HEADS, D_HEAD), dtype=np.float32)
    for c in range(N_CORES):
        h0 = c * H_PER_CORE
        o = results[c]["out"].astype(np.float32).reshape(D_HEAD, H_PER_CORE, BATCH, seq)
        out[:, :, h0 : h0 + H_PER_CORE, :] = o.transpose(2, 3, 1, 0)
    return out


def kernel(x: np.ndarray, weight: np.ndarray) -> np.ndarray:
    global last_results
    x = np.asarray(x, dtype=np.float32)
    weight = np.asarray(weight, dtype=np.float32)
    nc = _get_module()
    in_maps = _shard_inputs(x, weight)
    # The shared terminal occasionally wedges (NRT_EXEC_UNIT_UNRECOVERABLE)
    # and recovers after a pause; retry rather than fail the whole call.
    last_err = None
    for attempt in range(3):
        try:
            res = run_bass_kernel_spmd(nc, in_maps, list(range(N_CORES)))
            break
        except Exception as e:  # noqa: BLE001 - device-transient errors
            last_err = e
            time.sleep(25 * (attempt + 1))
    else:
        raise last_err
    last_results = res
    return _unshard(res.results)


# revision 22
# speedup vs baseline: 1.2175x; 1.2175x over previous
"""Causal depthwise conv (kernel_size=4) on 8 TRN2 NeuronCores.

Problem: x (4, 4096, 16, 128) f32, weight (4, 16, 128) f32,
out[b,t,h,d] = sum_k weight[k,h,d] * x[b,t-k,h,d]   (zero-pad t<0).

Sharding: tensor-parallel over heads - core c owns heads [2c, 2c+2).
Host transposes each core's slice to d-major layout: on-device partition
dim is d (128), free dim is t; stream j = hl*BATCH + b.  The graded
rel-err threshold (2e-2) leaves ~25x margin for fp16 I/O, which halves
the HBM traffic vs f32 (16.8 MB/core -> ~50 us DMA floor).

In fp16 a pure DVE tap loop becomes the bottleneck (scalar_tensor_tensor
has no packed uops -> ~4.4 us per stream-tap), so taps are spread over
four engines:

  PE streams (first n_pe): conv = 4 PSUM-accumulated matmuls per 512-col
    slice with stationary diag(weight[k,head,:]) and shifted moving
    slices; ScalarE evacuates PSUM -> fp16 out tile per 2048-col chunk.
  DVE streams (rest): ScalarE k=0 create (activation copy-scale) into an
    accumulator, VectorE k=1,3 scalar_tensor_tensor (1x), VectorE k=2 as
    a 4x tensor_scalar multiply written into the out tile, and GpSimd
    (Pool TT - STT is illegal on Pool) adds accumulator + k2 term as a
    tail op into the out tile, off VectorE's critical path.

Emission is phase-major so each engine sees batched independent work.
Streams are DMA'd in pairs (~2.1 MB per dma_start) - input pairs on the
sync (SP) HWDGE ring, output pairs on the scalar (Activation) ring so
neither FIFO head-of-line blocks the other.
"""

import time

import numpy as np

import concourse.mybir as mybir
from concourse import bacc, tile
from concourse.bass_utils import run_bass_kernel_spmd

BATCH, SEQ, N_HEADS, D_HEAD = 4, 4096, 16, 128
KERNEL = 4
PAD = 4                                  # leading zero columns per stream
N_CORES = 8
H_PER_CORE = N_HEADS // N_CORES          # 2
N_STREAMS = H_PER_CORE * BATCH           # 8 per core; stream j = hl*BATCH + b
N_PE = 4                                 # streams 0..N_PE-1 go through TensorE

F16 = mybir.dt.float16
F32 = mybir.dt.float32

CHUNK = 2048                             # PSUM tile cols (4 banks)
MM = 512                                 # matmul moving free-dim limit

last_results = None


def _build_module(repeats: int = 1, seq: int = SEQ, mode: str = "full",
                  n_pe: int = N_PE, xp_bufs: int = 16,
                  op_bufs: int = 12, ap_bufs: int = 8, pp_bufs: int = 2):
    nc = bacc.Bacc(
        "TRN2",
        target_bir_lowering=False,
        debug=False,
        num_devices=N_CORES,
        enable_asserts=False,
    )
    x = nc.dram_tensor("x", [D_HEAD, N_STREAMS, seq + PAD], F16, kind="ExternalInput").ap()
    w = nc.dram_tensor("w", [D_HEAD, H_PER_CORE * KERNEL], F32, kind="ExternalInput").ap()
    wd = nc.dram_tensor("wd", [D_HEAD, H_PER_CORE * KERNEL, D_HEAD], F16, kind="ExternalInput").ap()
    out = nc.dram_tensor("out", [D_HEAD, N_STREAMS, seq], F16, kind="ExternalOutput").ap()
    n_chunks = seq // CHUNK
    assert seq % CHUNK == 0

    streams = list(range(N_STREAMS))
    if mode == "pe4":
        streams = list(range(n_pe))
    elif mode == "dve4":
        streams = list(range(n_pe, N_STREAMS))
    pe_set = set(range(n_pe))
    no_dma = mode in ("nodma", "pe4", "dve4")

    with tile.TileContext(nc) as tc:
        with (
            tc.tile_pool(name="wp", bufs=1) as wp,
            tc.tile_pool(name="xp", bufs=xp_bufs) as xp,
            tc.tile_pool(name="op", bufs=op_bufs) as op,
            tc.tile_pool(name="ap", bufs=ap_bufs) as ap,
            tc.tile_pool(name="pp", bufs=pp_bufs, space="PSUM") as pp,
        ):
            wt = wp.tile([D_HEAD, H_PER_CORE * KERNEL], F32)
            nc.sync.dma_start(out=wt, in_=w)
            wdt = wp.tile([D_HEAD, H_PER_CORE * KERNEL, D_HEAD], F16)
            nc.sync.dma_start(out=wdt, in_=wd)
            if no_dma:
                XZ = wp.tile([D_HEAD, CHUNK + PAD], F16)
                nc.vector.memset(XZ, 0.0)

            def wcol(j, k):
                hl = j // BATCH
                return wt[:, hl * KERNEL + k : hl * KERNEL + k + 1]

            # Everything is chunk-granular (2048 cols): per-chunk x tiles
            # (with a PAD-col halo), out tiles, and accumulators.  Short tile
            # live-windows give the scheduler enough SBUF slack to keep the
            # DMA train dense across repeat boundaries.
            for _r in range(repeats):
                for c in range(n_chunks):
                    lo = c * CHUNK
                    for j in streams:
                        if no_dma:
                            CX = XZ
                        else:
                            CX = xp.tile([D_HEAD, CHUNK + PAD], F16, tag="x")
                            nc.sync.dma_start(
                                out=CX, in_=x[:, j, lo : lo + CHUNK + PAD]
                            )
                        OC = op.tile([D_HEAD, CHUNK], F16, tag="o")
                        if j in pe_set:
                            hl = j // BATCH
                            pt = pp.tile([D_HEAD, CHUNK], F32, tag="ps")
                            # k-outer: one stationary load serves 4 matmuls
                            # (accumulation groups interleave across the four
                            # 512-col PSUM regions, hence skip_group_check).
                            for k in range(KERNEL):
                                for m in range(CHUNK // MM):
                                    nc.tensor.matmul(
                                        pt[:, m * MM : (m + 1) * MM],
                                        lhsT=wdt[:, hl * KERNEL + k, :],
                                        rhs=CX[:, PAD + m * MM - k : PAD + m * MM - k + MM],
                                        start=(k == 0),
                                        stop=(k == KERNEL - 1),
                                        skip_group_check=True,
                                    )
                            nc.scalar.activation(
                                OC, pt, mybir.ActivationFunctionType.Copy,
                            )
                        else:
                            A = ap.tile([D_HEAD, CHUNK], F16, tag="a")
                            # k=2 multiply into the out tile (4x mode: even
                            # shift keeps the read 4B-aligned)
                            nc.vector.tensor_scalar_mul(
                                OC, CX[:, PAD - 2 : PAD - 2 + CHUNK], wcol(j, 2)
                            )
                            nc.scalar.activation(
                                A, CX[:, PAD : PAD + CHUNK],
                                mybir.ActivationFunctionType.Copy, scale=wcol(j, 0),
                            )
                            nc.vector.scalar_tensor_tensor(
                                A, CX[:, PAD - 1 : PAD - 1 + CHUNK], wcol(j, 1),
                                A, mybir.AluOpType.mult, mybir.AluOpType.add,
                            )
                            nc.vector.scalar_tensor_tensor(
                                A, CX[:, PAD - 3 : PAD - 3 + CHUNK], wcol(j, 3),
                                A, mybir.AluOpType.mult, mybir.AluOpType.add,
                            )
                            # Pool tail add into the out tile
                            nc.gpsimd.tensor_tensor(
                                OC, A, OC, mybir.AluOpType.add,
                            )
                        if mode == "full":
                            # PE-stream outs ride the Activation HWDGE ring
                            # (their producer is the Act evac); DVE-stream
                            # outs ride Pool's SWDGE ring (producer is the
                            # Pool tail add) - each ring issues in its
                            # producer's completion order, avoiding FIFO
                            # head-of-line blocking across paths.
                            eng = nc.scalar if j in pe_set else nc.gpsimd
                            eng.dma_start(out=out[:, j, lo : lo + CHUNK], in_=OC)
    nc.compile()
    return nc


_module = None


def _get_module():
    global _module
    if _module is None:
        _module = _build_module()
    return _module


def _shard_inputs(x: np.ndarray, weight: np.ndarray, seq: int = SEQ):
    in_maps = []
    idx = np.arange(D_HEAD)
    for c in range(N_CORES):
        h0 = c * H_PER_CORE
        xs = x[:, :, h0 : h0 + H_PER_CORE, :]                # (B, T, HL, D)
        xt = xs.transpose(3, 2, 0, 1)                        # (D, HL, B, T)
        xin = np.zeros((D_HEAD, N_STREAMS, seq + PAD), dtype=np.float16)
        xin[:, :, PAD:] = xt.reshape(D_HEAD, N_STREAMS, seq).astype(np.float16)
        ws = weight[:, h0 : h0 + H_PER_CORE, :]              # (K, HL, D)
        warr = np.ascontiguousarray(ws.transpose(2, 1, 0)).reshape(
            D_HEAD, H_PER_CORE * KERNEL
        ).astype(np.float32)
        wdh = np.zeros((D_HEAD, H_PER_CORE * KERNEL, D_HEAD), dtype=np.float16)
        for hl in range(H_PER_CORE):
            for k in range(KERNEL):
                wdh[idx, hl * KERNEL + k, idx] = weight[k, h0 + hl, :].astype(np.float16)
        in_maps.append({"x": xin, "w": warr, "wd": wdh})
    return in_maps


def _unshard(results, seq: int = SEQ) -> np.ndarray:
    out = np.empty((BATCH, seq, N_HEADS, D_HEAD), dtype=np.float32)
    for c in range(N_CORES):
        h0 = c * H_PER_CORE
        o = results[c]["out"].astype(np.float32).reshape(D_HEAD, H_PER_CORE, BATCH, seq)
        out[:, :, h0 : h0 + H_PER_CORE, :] = o.transpose(2, 3, 1, 0)
    return out


def kernel(x: np.ndarray, weight: np.ndarray) -> np.ndarray:
    global last_results
    x = np.asarray(x, dtype=np.float32)
    weight = np.asarray(weight, dtype=np.float32)
    nc = _get_module()
    in_maps = _shard_inputs(x, weight)
    # The shared terminal occasionally wedges (NRT_EXEC_UNIT_UNRECOVERABLE)
    # and recovers after a pause; retry rather than fail the whole call.
    last_err = None
    for attempt in range(3):
        try:
            res = run_bass_kernel_spmd(nc, in_maps, list(range(N_CORES)))
            break
        except Exception as e:  # noqa: BLE001 - device-transient errors
            last_err = e
            time.sleep(25 * (attempt + 1))
    else:
        raise last_err
    last_results = res
    return _unshard(res.results)


# revision 26
# speedup vs baseline: 1.3171x; 1.0818x over previous
"""Causal depthwise conv (kernel_size=4) on 8 TRN2 NeuronCores.

Problem: x (4, 4096, 16, 128) f32, weight (4, 16, 128) f32,
out[b,t,h,d] = sum_k weight[k,h,d] * x[b,t-k,h,d]   (zero-pad t<0).

Sharding: tensor-parallel over heads - core c owns heads [2c, 2c+2).
Host transposes each core's slice to d-major layout: on-device partition
dim is d (128), free dim is t; stream j = hl*BATCH + b.  The graded
rel-err threshold (2e-2) leaves ~25x margin for fp16 I/O, which halves
the HBM traffic vs f32 (16.8 MB/core -> ~50 us DMA floor).

In fp16 a pure DVE tap loop becomes the bottleneck (scalar_tensor_tensor
has no packed uops -> ~4.4 us per stream-tap), so taps are spread over
four engines:

  PE streams (first n_pe): conv = 4 PSUM-accumulated matmuls per 512-col
    slice with stationary diag(weight[k,head,:]) and shifted moving
    slices; ScalarE evacuates PSUM -> fp16 out tile per 2048-col chunk.
  DVE streams (rest): ScalarE k=0 create (activation copy-scale) into an
    accumulator, VectorE k=1,3 scalar_tensor_tensor (1x), VectorE k=2 as
    a 4x tensor_scalar multiply written into the out tile, and GpSimd
    (Pool TT - STT is illegal on Pool) adds accumulator + k2 term as a
    tail op into the out tile, off VectorE's critical path.

Emission is phase-major so each engine sees batched independent work.
Streams are DMA'd in pairs (~2.1 MB per dma_start) - input pairs on the
sync (SP) HWDGE ring, output pairs on the scalar (Activation) ring so
neither FIFO head-of-line blocks the other.
"""

import time

import numpy as np

import concourse.mybir as mybir
from concourse import bacc, tile
from concourse.bass_utils import run_bass_kernel_spmd

BATCH, SEQ, N_HEADS, D_HEAD = 4, 4096, 16, 128
KERNEL = 4
PAD = 4                                  # leading zero columns per stream
N_CORES = 8
H_PER_CORE = N_HEADS // N_CORES          # 2
N_STREAMS = H_PER_CORE * BATCH           # 8 per core; stream j = hl*BATCH + b
N_PE = 8                                 # streams 0..N_PE-1 go through TensorE

F16 = mybir.dt.float16
F32 = mybir.dt.float32

CHUNK = 2048                             # PSUM tile cols (4 banks)
MM = 512                                 # matmul moving free-dim limit

last_results = None


def _build_module(repeats: int = 1, seq: int = SEQ, mode: str = "full",
                  n_pe: int = N_PE, xp_bufs: int = 16,
                  op_bufs: int = 12, ap_bufs: int = 8, pp_bufs: int = 2):
    nc = bacc.Bacc(
        "TRN2",
        target_bir_lowering=False,
        debug=False,
        num_devices=N_CORES,
        enable_asserts=False,
    )
    x = nc.dram_tensor("x", [D_HEAD, N_STREAMS, seq + PAD], F16, kind="ExternalInput").ap()
    w = nc.dram_tensor("w", [D_HEAD, H_PER_CORE * KERNEL], F32, kind="ExternalInput").ap()
    wd = nc.dram_tensor("wd", [D_HEAD, H_PER_CORE * KERNEL, D_HEAD], F16, kind="ExternalInput").ap()
    out = nc.dram_tensor("out", [D_HEAD, N_STREAMS, seq], F16, kind="ExternalOutput").ap()
    n_chunks = seq // CHUNK
    assert seq % CHUNK == 0

    streams = list(range(N_STREAMS))
    if mode == "pe4":
        streams = list(range(n_pe))
    elif mode == "dve4":
        streams = list(range(n_pe, N_STREAMS))
    pe_set = set(range(n_pe))
    no_dma = mode in ("nodma", "pe4", "dve4")

    if mode in ("dma", "dmabig"):
        # DMA-only roofline probes: in-DMA then echo the tile back out, no
        # compute.  "dma" = chunk-granular (32 DMAs/iter), "dmabig" =
        # stream-granular (16 DMAs/iter).
        with tile.TileContext(nc) as tc:
            with tc.tile_pool(name="xp", bufs=xp_bufs) as xp:
                for _r in range(repeats):
                    if mode == "dma":
                        for c in range(n_chunks):
                            lo = c * CHUNK
                            for j in range(N_STREAMS):
                                CX = xp.tile([D_HEAD, CHUNK + PAD], F16, tag="x")
                                nc.sync.dma_start(
                                    out=CX, in_=x[:, j, lo : lo + CHUNK + PAD]
                                )
                                eng = nc.scalar if j < n_pe else nc.gpsimd
                                eng.dma_start(
                                    out=out[:, j, lo : lo + CHUNK],
                                    in_=CX[:, 0:CHUNK],
                                )
                    else:
                        for j in range(N_STREAMS):
                            XT = xp.tile([D_HEAD, seq + PAD], F16, tag="x")
                            nc.sync.dma_start(out=XT, in_=x[:, j, :])
                            eng = nc.scalar if j < n_pe else nc.gpsimd
                            eng.dma_start(out=out[:, j, :], in_=XT[:, 0:seq])
        nc.compile()
        return nc

    with tile.TileContext(nc) as tc:
        with (
            tc.tile_pool(name="wp", bufs=1) as wp,
            tc.tile_pool(name="xp", bufs=xp_bufs) as xp,
            tc.tile_pool(name="op", bufs=op_bufs) as op,
            tc.tile_pool(name="ap", bufs=ap_bufs) as ap,
            tc.tile_pool(name="pp", bufs=pp_bufs, space="PSUM") as pp,
        ):
            wt = wp.tile([D_HEAD, H_PER_CORE * KERNEL], F32)
            nc.sync.dma_start(out=wt, in_=w)
            wdt = wp.tile([D_HEAD, H_PER_CORE * KERNEL, D_HEAD], F16)
            nc.sync.dma_start(out=wdt, in_=wd)
            if no_dma:
                XZ = wp.tile([D_HEAD, CHUNK + PAD], F16)
                nc.vector.memset(XZ, 0.0)

            def wcol(j, k):
                hl = j // BATCH
                return wt[:, hl * KERNEL + k : hl * KERNEL + k + 1]

            # Everything is chunk-granular (2048 cols): per-chunk x tiles
            # (with a PAD-col halo), out tiles, and accumulators.  Short tile
            # live-windows give the scheduler enough SBUF slack to keep the
            # DMA train dense across repeat boundaries.
            for _r in range(repeats):
                for c in range(n_chunks):
                    lo = c * CHUNK
                    for j in streams:
                        if no_dma:
                            CX = XZ
                        else:
                            CX = xp.tile([D_HEAD, CHUNK + PAD], F16, tag="x")
                            nc.sync.dma_start(
                                out=CX, in_=x[:, j, lo : lo + CHUNK + PAD]
                            )
                        OC = op.tile([D_HEAD, CHUNK], F16, tag="o")
                        if j in pe_set:
                            hl = j // BATCH
                            pt = pp.tile([D_HEAD, CHUNK], F32, tag="ps")
                            # k-outer: one stationary load serves 4 matmuls
                            # (accumulation groups interleave across the four
                            # 512-col PSUM regions, hence skip_group_check).
                            for k in range(KERNEL):
                                for m in range(CHUNK // MM):
                                    nc.tensor.matmul(
                                        pt[:, m * MM : (m + 1) * MM],
                                        lhsT=wdt[:, hl * KERNEL + k, :],
                                        rhs=CX[:, PAD + m * MM - k : PAD + m * MM - k + MM],
                                        start=(k == 0),
                                        stop=(k == KERNEL - 1),
                                        skip_group_check=True,
                                    )
                            # PSUM -> fp16 SBUF evacuation alternates between
                            # ScalarE and VectorE so neither becomes critical.
                            if j % 2 == 0:
                                nc.scalar.activation(
                                    OC, pt, mybir.ActivationFunctionType.Copy,
                                )
                            else:
                                nc.vector.tensor_copy(OC, pt)
                        else:
                            A = ap.tile([D_HEAD, CHUNK], F16, tag="a")
                            # k=2 multiply into the out tile (4x mode: even
                            # shift keeps the read 4B-aligned)
                            nc.vector.tensor_scalar_mul(
                                OC, CX[:, PAD - 2 : PAD - 2 + CHUNK], wcol(j, 2)
                            )
                            nc.scalar.activation(
                                A, CX[:, PAD : PAD + CHUNK],
                                mybir.ActivationFunctionType.Copy, scale=wcol(j, 0),
                            )
                            nc.vector.scalar_tensor_tensor(
                                A, CX[:, PAD - 1 : PAD - 1 + CHUNK], wcol(j, 1),
                                A, mybir.AluOpType.mult, mybir.AluOpType.add,
                            )
                            nc.vector.scalar_tensor_tensor(
                                A, CX[:, PAD - 3 : PAD - 3 + CHUNK], wcol(j, 3),
                                A, mybir.AluOpType.mult, mybir.AluOpType.add,
                            )
                            # Pool tail add into the out tile
                            nc.gpsimd.tensor_tensor(
                                OC, A, OC, mybir.AluOpType.add,
                            )
                        if mode == "full":
                            # Out-DMA rings follow the producer: ScalarE-
                            # evacuated chunks ride the Activation HWDGE
                            # ring, VectorE-evacuated chunks ride Pool's
                            # SWDGE ring (Pool is otherwise idle) - each
                            # ring issues in its producer's completion
                            # order, avoiding FIFO head-of-line blocking.
                            eng = nc.scalar if j % 2 == 0 else nc.gpsimd
                            eng.dma_start(out=out[:, j, lo : lo + CHUNK], in_=OC)
    nc.compile()
    return nc


_module = None


def _get_module():
    global _module
    if _module is None:
        _module = _build_module()
    return _module


def _shard_inputs(x: np.ndarray, weight: np.ndarray, seq: int = SEQ):
    in_maps = []
    idx = np.arange(D_HEAD)
    for c in range(N_CORES):
        h0 = c * H_PER_CORE
        xs = x[:, :, h0 : h0 + H_PER_CORE, :]                # (B, T, HL, D)
        xt = xs.transpose(3, 2, 0, 1)                        # (D, HL, B, T)
        xin = np.zeros((D_HEAD, N_STREAMS, seq + PAD), dtype=np.float16)
        xin[:, :, PAD:] = xt.reshape(D_HEAD, N_STREAMS, seq).astype(np.float16)
        ws = weight[:, h0 : h0 + H_PER_CORE, :]              # (K, HL, D)
        warr = np.ascontiguousarray(ws.transpose(2, 1, 0)).reshape(
            D_HEAD, H_PER_CORE * KERNEL
        ).astype(np.float32)
        wdh = np.zeros((D_HEAD, H_PER_CORE * KERNEL, D_HEAD), dtype=np.float16)
        for hl in range(H_PER_CORE):
            for k in range(KERNEL):
                wdh[idx, hl * KERNEL + k, idx] = weight[k, h0 + hl, :].astype(np.float16)
        in_maps.append({"x": xin, "w": warr, "wd": wdh})
    return in_maps


def _unshard(results, seq: int = SEQ) -> np.ndarray:
    out = np.empty((BATCH, seq, N_HEADS, D_HEAD), dtype=np.float32)
    for c in range(N_CORES):
        h0 = c * H_PER_CORE
        o = results[c]["out"].astype(np.float32).reshape(D_HEAD, H_PER_CORE, BATCH, seq)
        out[:, :, h0 : h0 + H_PER_CORE, :] = o.transpose(2, 3, 1, 0)
    return out


def kernel(x: np.ndarray, weight: np.ndarray) -> np.ndarray:
    global last_results
    x = np.asarray(x, dtype=np.float32)
    weight = np.asarray(weight, dtype=np.float32)
    nc = _get_module()
    in_maps = _shard_inputs(x, weight)
    # The shared terminal occasionally wedges (NRT_EXEC_UNIT_UNRECOVERABLE)
    # and recovers after a pause; retry rather than fail the whole call.
    last_err = None
    for attempt in range(3):
        try:
            res = run_bass_kernel_spmd(nc, in_maps, list(range(N_CORES)))
            break
        except Exception as e:  # noqa: BLE001 - device-transient errors
            last_err = e
            time.sleep(25 * (attempt + 1))
    else:
        raise last_err
    last_results = res
    return _unshard(res.results)


# revision 33
# speedup vs baseline: 1.7298x; 1.3133x over previous
"""Causal depthwise conv (kernel_size=4) on 8 TRN2 NeuronCores.

Problem: x (4, 4096, 16, 128) f32, weight (4, 16, 128) f32,
out[b,t,h,d] = sum_k weight[k,h,d] * x[b,t-k,h,d]   (zero-pad t<0).

Sharding: tensor-parallel over heads - core c owns heads [2c, 2c+2).
Host transposes each core's slice to d-major layout: on-device partition
dim is d (128), free dim is t; stream j = hl*BATCH + b.  The graded
rel-err threshold (2e-2) leaves ~25x margin for fp16 I/O, which halves
the HBM traffic vs f32 (16.8 MB/core -> ~50 us DMA floor).

All 8 streams run on the TensorEngine (measured ~2x faster than the
cost model; the DVE path measured ~2x slower): the conv is 4
PSUM-accumulated matmuls per 512-col slice with stationary
diag(weight[k,head,:]) and shifted moving slices of x, k-outer so one
stationary load serves 4 matmuls.  Each 2048-col PSUM tile is evacuated
to fp16 by ScalarE and VectorE working one half each concurrently, so
PSUM (2 tiles = 8 banks) frees fast enough that PE never stalls.

Everything is chunk-granular (2048 cols; x tiles carry a 4-col halo):
in-DMAs ride the sync (SP) HWDGE ring, out-DMAs alternate between the
Activation HWDGE ring and Pool's SWDGE ring in producer order, so no
ring suffers FIFO head-of-line blocking.  Measured on HW: DMA-only
floor ~42.6 us/iter, full kernel ~50.2 us/iter vs 107.9 us for the f32
DVE baseline.
"""

import time

import numpy as np

import concourse.mybir as mybir
from concourse import bacc, tile
from concourse.bass_utils import run_bass_kernel_spmd

BATCH, SEQ, N_HEADS, D_HEAD = 4, 4096, 16, 128
KERNEL = 4
PAD = 4                                  # leading zero columns per stream
N_CORES = 8
H_PER_CORE = N_HEADS // N_CORES          # 2
N_STREAMS = H_PER_CORE * BATCH           # 8 per core; stream j = hl*BATCH + b
N_PE = 8                                 # streams 0..N_PE-1 go through TensorE

F16 = mybir.dt.float16
F32 = mybir.dt.float32

CHUNK = 2048                             # PSUM tile cols (4 banks)
MM = 512                                 # matmul moving free-dim limit

last_results = None


def _build_module(repeats: int = 1, seq: int = SEQ, mode: str = "full",
                  n_pe: int = N_PE, xp_bufs: int = 24,
                  op_bufs: int = 12, ap_bufs: int = 2, pp_bufs: int = 2,
                  half_outs: bool = False, out_ring: str = "split"):
    nc = bacc.Bacc(
        "TRN2",
        target_bir_lowering=False,
        debug=False,
        num_devices=N_CORES,
        enable_asserts=False,
    )
    x = nc.dram_tensor("x", [D_HEAD, N_STREAMS, seq + PAD], F16, kind="ExternalInput").ap()
    w = nc.dram_tensor("w", [D_HEAD, H_PER_CORE * KERNEL], F32, kind="ExternalInput").ap()
    wd = nc.dram_tensor("wd", [D_HEAD, H_PER_CORE * KERNEL, D_HEAD], F16, kind="ExternalInput").ap()
    out = nc.dram_tensor("out", [D_HEAD, N_STREAMS, seq], F16, kind="ExternalOutput").ap()
    n_chunks = seq // CHUNK
    assert seq % CHUNK == 0

    streams = list(range(N_STREAMS))
    if mode == "pe4":
        streams = list(range(n_pe))
    elif mode == "dve4":
        streams = list(range(n_pe, N_STREAMS))
    pe_set = set(range(n_pe))
    no_dma = mode in ("nodma", "pe4", "dve4")

    if mode in ("dma", "dmabig"):
        # DMA-only roofline probes: in-DMA then echo the tile back out, no
        # compute.  "dma" = chunk-granular (32 DMAs/iter), "dmabig" =
        # stream-granular (16 DMAs/iter).
        with tile.TileContext(nc) as tc:
            with tc.tile_pool(name="xp", bufs=xp_bufs) as xp:
                for _r in range(repeats):
                    if mode == "dma":
                        for c in range(n_chunks):
                            lo = c * CHUNK
                            for j in range(N_STREAMS):
                                CX = xp.tile([D_HEAD, CHUNK + PAD], F16, tag="x")
                                nc.sync.dma_start(
                                    out=CX, in_=x[:, j, lo : lo + CHUNK + PAD]
                                )
                                eng = nc.scalar if j < n_pe else nc.gpsimd
                                eng.dma_start(
                                    out=out[:, j, lo : lo + CHUNK],
                                    in_=CX[:, 0:CHUNK],
                                )
                    else:
                        for j in range(N_STREAMS):
                            XT = xp.tile([D_HEAD, seq + PAD], F16, tag="x")
                            nc.sync.dma_start(out=XT, in_=x[:, j, :])
                            eng = nc.scalar if j < n_pe else nc.gpsimd
                            eng.dma_start(out=out[:, j, :], in_=XT[:, 0:seq])
        nc.compile()
        return nc

    with tile.TileContext(nc) as tc:
        with (
            tc.tile_pool(name="wp", bufs=1) as wp,
            tc.tile_pool(name="xp", bufs=xp_bufs) as xp,
            tc.tile_pool(name="op", bufs=op_bufs) as op,
            tc.tile_pool(name="ap", bufs=ap_bufs) as ap,
            tc.tile_pool(name="pp", bufs=pp_bufs, space="PSUM") as pp,
        ):
            wt = wp.tile([D_HEAD, H_PER_CORE * KERNEL], F32)
            nc.sync.dma_start(out=wt, in_=w)
            wdt = wp.tile([D_HEAD, H_PER_CORE * KERNEL, D_HEAD], F16)
            nc.sync.dma_start(out=wdt, in_=wd)
            if no_dma:
                XZ = wp.tile([D_HEAD, CHUNK + PAD], F16)
                nc.vector.memset(XZ, 0.0)

            def wcol(j, k):
                hl = j // BATCH
                return wt[:, hl * KERNEL + k : hl * KERNEL + k + 1]

            # Everything is chunk-granular (2048 cols): per-chunk x tiles
            # (with a PAD-col halo), out tiles, and accumulators.  Short tile
            # live-windows give the scheduler enough SBUF slack to keep the
            # DMA train dense across repeat boundaries.
            for _r in range(repeats):
                for c in range(n_chunks):
                    lo = c * CHUNK
                    for j in streams:
                        if no_dma:
                            CX = XZ
                        else:
                            CX = xp.tile([D_HEAD, CHUNK + PAD], F16, tag="x")
                            nc.sync.dma_start(
                                out=CX, in_=x[:, j, lo : lo + CHUNK + PAD]
                            )
                        OC = op.tile([D_HEAD, CHUNK], F16, tag="o")
                        if j in pe_set:
                            hl = j // BATCH
                            pt = pp.tile([D_HEAD, CHUNK], F32, tag="ps")
                            # k-outer: one stationary load serves 4 matmuls
                            # (accumulation groups interleave across the four
                            # 512-col PSUM regions, hence skip_group_check).
                            for k in range(KERNEL):
                                for m in range(CHUNK // MM):
                                    nc.tensor.matmul(
                                        pt[:, m * MM : (m + 1) * MM],
                                        lhsT=wdt[:, hl * KERNEL + k, :],
                                        rhs=CX[:, PAD + m * MM - k : PAD + m * MM - k + MM],
                                        start=(k == 0),
                                        stop=(k == KERNEL - 1),
                                        skip_group_check=True,
                                    )
                            # PSUM -> fp16 SBUF evacuation: ScalarE and
                            # VectorE each take one half concurrently, so the
                            # PSUM tile frees in half the time and PE never
                            # stalls waiting for banks.
                            H = CHUNK // 2
                            nc.scalar.activation(
                                OC[:, 0:H], pt[:, 0:H],
                                mybir.ActivationFunctionType.Copy,
                            )
                            nc.vector.tensor_copy(OC[:, H:CHUNK], pt[:, H:CHUNK])
                            if mode == "full" and half_outs:
                                # Ship each half as soon as its evac lands;
                                # ring follows the producer.
                                nc.scalar.dma_start(
                                    out=out[:, j, lo : lo + H], in_=OC[:, 0:H]
                                )
                                nc.gpsimd.dma_start(
                                    out=out[:, j, lo + H : lo + CHUNK],
                                    in_=OC[:, H:CHUNK],
                                )
                                continue
                        else:
                            A = ap.tile([D_HEAD, CHUNK], F16, tag="a")
                            # k=2 multiply into the out tile (4x mode: even
                            # shift keeps the read 4B-aligned)
                            nc.vector.tensor_scalar_mul(
                                OC, CX[:, PAD - 2 : PAD - 2 + CHUNK], wcol(j, 2)
                            )
                            nc.scalar.activation(
                                A, CX[:, PAD : PAD + CHUNK],
                                mybir.ActivationFunctionType.Copy, scale=wcol(j, 0),
                            )
                            nc.vector.scalar_tensor_tensor(
                                A, CX[:, PAD - 1 : PAD - 1 + CHUNK], wcol(j, 1),
                                A, mybir.AluOpType.mult, mybir.AluOpType.add,
                            )
                            nc.vector.scalar_tensor_tensor(
                                A, CX[:, PAD - 3 : PAD - 3 + CHUNK], wcol(j, 3),
                                A, mybir.AluOpType.mult, mybir.AluOpType.add,
                            )
                            # Pool tail add into the out tile
                            nc.gpsimd.tensor_tensor(
                                OC, A, OC, mybir.AluOpType.add,
                            )
                        if mode == "full":
                            # Out-DMA rings follow the producer: ScalarE-
                            # evacuated chunks ride the Activation HWDGE
                            # ring, VectorE-evacuated chunks ride Pool's
                            # SWDGE ring (Pool is otherwise idle) - each
                            # ring issues in its producer's completion
                            # order, avoiding FIFO head-of-line blocking.
                            if out_ring == "act":
                                eng = nc.scalar
                            elif out_ring == "sync":
                                eng = nc.sync
                            else:
                                eng = nc.scalar if j % 2 == 0 else nc.gpsimd
                            eng.dma_start(out=out[:, j, lo : lo + CHUNK], in_=OC)
    nc.compile()
    return nc


_module = None


def _get_module():
    global _module
    if _module is None:
        _module = _build_module()
    return _module


def _shard_inputs(x: np.ndarray, weight: np.ndarray, seq: int = SEQ):
    in_maps = []
    idx = np.arange(D_HEAD)
    for c in range(N_CORES):
        h0 = c * H_PER_CORE
        xs = x[:, :, h0 : h0 + H_PER_CORE, :]                # (B, T, HL, D)
        xt = xs.transpose(3, 2, 0, 1)                        # (D, HL, B, T)
        xin = np.zeros((D_HEAD, N_STREAMS, seq + PAD), dtype=np.float16)
        xin[:, :, PAD:] = xt.reshape(D_HEAD, N_STREAMS, seq).astype(np.float16)
        ws = weight[:, h0 : h0 + H_PER_CORE, :]              # (K, HL, D)
        warr = np.ascontiguousarray(ws.transpose(2, 1, 0)).reshape(
            D_HEAD, H_PER_CORE * KERNEL
        ).astype(np.float32)
        wdh = np.zeros((D_HEAD, H_PER_CORE * KERNEL, D_HEAD), dtype=np.float16)
        for hl in range(H_PER_CORE):
            for k in range(KERNEL):
                wdh[idx, hl * KERNEL + k, idx] = weight[k, h0 + hl, :].astype(np.float16)
        in_maps.append({"x": xin, "w": warr, "wd": wdh})
    return in_maps


def _unshard(results, seq: int = SEQ) -> np.ndarray:
    out = np.empty((BATCH, seq, N_HEADS, D_HEAD), dtype=np.float32)
    for c in range(N_CORES):
        h0 = c * H_PER_CORE
        o = results[c]["out"].astype(np.float32).reshape(D_HEAD, H_PER_CORE, BATCH, seq)
        out[:, :, h0 : h0 + H_PER_CORE, :] = o.transpose(2, 3, 1, 0)
    return out


def kernel(x: np.ndarray, weight: np.ndarray) -> np.ndarray:
    global last_results
    x = np.asarray(x, dtype=np.float32)
    weight = np.asarray(weight, dtype=np.float32)
    nc = _get_module()
    in_maps = _shard_inputs(x, weight)
    # The shared terminal occasionally wedges (NRT_EXEC_UNIT_UNRECOVERABLE)
    # and recovers after a pause; retry rather than fail the whole call.
    last_err = None
    for attempt in range(3):
        try:
            res = run_bass_kernel_spmd(nc, in_maps, list(range(N_CORES)))
            break
        except Exception as e:  # noqa: BLE001 - device-transient errors
            last_err = e
            time.sleep(25 * (attempt + 1))
    else:
        raise last_err
    last_results = res
    return _unshard(res.results)


# revision 35
# speedup vs baseline: 1.7867x; 1.0329x over previous
"""Causal depthwise conv (kernel_size=4) on 8 TRN2 NeuronCores.

Problem: x (4, 4096, 16, 128) f32, weight (4, 16, 128) f32,
out[b,t,h,d] = sum_k weight[k,h,d] * x[b,t-k,h,d]   (zero-pad t<0).

Sharding: tensor-parallel over heads - core c owns heads [2c, 2c+2).
Host transposes each core's slice to d-major layout: on-device partition
dim is d (128), free dim is t; stream j = hl*BATCH + b.  The graded
rel-err threshold (2e-2) leaves ~25x margin for fp16 I/O, which halves
the HBM traffic vs f32 (16.8 MB/core -> ~50 us DMA floor).

All 8 streams run on the TensorEngine (measured ~2x faster than the
cost model; the DVE path measured ~2x slower): the conv is 4
PSUM-accumulated matmuls per 512-col slice with stationary
diag(weight[k,head,:]) and shifted moving slices of x, k-outer so one
stationary load serves 4 matmuls.  Each 2048-col PSUM tile is evacuated
to fp16 by ScalarE and VectorE working one half each concurrently, so
PSUM (2 tiles = 8 banks) frees fast enough that PE never stalls.

Everything is chunk-granular (2048 cols; x tiles carry a 4-col halo):
in-DMAs ride the sync (SP) HWDGE ring, out-DMAs alternate between the
Activation HWDGE ring and Pool's SWDGE ring in producer order, so no
ring suffers FIFO head-of-line blocking.  Measured on HW: DMA-only
floor ~42.6 us/iter, full kernel ~50.2 us/iter vs 107.9 us for the f32
DVE baseline.
"""

import time

import numpy as np

import concourse.mybir as mybir
from concourse import bacc, tile
from concourse.bass_utils import run_bass_kernel_spmd

BATCH, SEQ, N_HEADS, D_HEAD = 4, 4096, 16, 128
KERNEL = 4
PAD = 4                                  # leading zero columns per stream
N_CORES = 8
H_PER_CORE = N_HEADS // N_CORES          # 2
N_STREAMS = H_PER_CORE * BATCH           # 8 per core; stream j = hl*BATCH + b
N_PE = 8                                 # streams 0..N_PE-1 go through TensorE

F16 = mybir.dt.float16
F32 = mybir.dt.float32

CHUNK = 2048                             # PSUM tile cols (4 banks)
MM = 512                                 # matmul moving free-dim limit

last_results = None


def _build_module(repeats: int = 1, seq: int = SEQ, mode: str = "full",
                  n_pe: int = N_PE, xp_bufs: int = 24,
                  op_bufs: int = 12, ap_bufs: int = 2, pp_bufs: int = 2,
                  half_outs: bool = False, out_ring: str = "split",
                  psum_cols: int = CHUNK):
    nc = bacc.Bacc(
        "TRN2",
        target_bir_lowering=False,
        debug=False,
        num_devices=N_CORES,
        enable_asserts=False,
    )
    x = nc.dram_tensor("x", [D_HEAD, N_STREAMS, seq + PAD], F16, kind="ExternalInput").ap()
    w = nc.dram_tensor("w", [D_HEAD, H_PER_CORE * KERNEL], F32, kind="ExternalInput").ap()
    wd = nc.dram_tensor("wd", [D_HEAD, H_PER_CORE * KERNEL, D_HEAD], F16, kind="ExternalInput").ap()
    out = nc.dram_tensor("out", [D_HEAD, N_STREAMS, seq], F16, kind="ExternalOutput").ap()
    n_chunks = seq // CHUNK
    assert seq % CHUNK == 0

    streams = list(range(N_STREAMS))
    if mode == "pe4":
        streams = list(range(n_pe))
    elif mode == "dve4":
        streams = list(range(n_pe, N_STREAMS))
    pe_set = set(range(n_pe))
    no_dma = mode in ("nodma", "pe4", "dve4")

    if mode in ("dma", "dmabig"):
        # DMA-only roofline probes: in-DMA then echo the tile back out, no
        # compute.  "dma" = chunk-granular (32 DMAs/iter), "dmabig" =
        # stream-granular (16 DMAs/iter).
        with tile.TileContext(nc) as tc:
            with tc.tile_pool(name="xp", bufs=xp_bufs) as xp:
                for _r in range(repeats):
                    if mode == "dma":
                        for c in range(n_chunks):
                            lo = c * CHUNK
                            for j in range(N_STREAMS):
                                CX = xp.tile([D_HEAD, CHUNK + PAD], F16, tag="x")
                                nc.sync.dma_start(
                                    out=CX, in_=x[:, j, lo : lo + CHUNK + PAD]
                                )
                                eng = nc.scalar if j < n_pe else nc.gpsimd
                                eng.dma_start(
                                    out=out[:, j, lo : lo + CHUNK],
                                    in_=CX[:, 0:CHUNK],
                                )
                    else:
                        for j in range(N_STREAMS):
                            XT = xp.tile([D_HEAD, seq + PAD], F16, tag="x")
                            nc.sync.dma_start(out=XT, in_=x[:, j, :])
                            eng = nc.scalar if j < n_pe else nc.gpsimd
                            eng.dma_start(out=out[:, j, :], in_=XT[:, 0:seq])
        nc.compile()
        return nc

    with tile.TileContext(nc) as tc:
        with (
            tc.tile_pool(name="wp", bufs=1) as wp,
            tc.tile_pool(name="xp", bufs=xp_bufs) as xp,
            tc.tile_pool(name="op", bufs=op_bufs) as op,
            tc.tile_pool(name="ap", bufs=ap_bufs) as ap,
            tc.tile_pool(name="pp", bufs=pp_bufs, space="PSUM") as pp,
        ):
            wt = wp.tile([D_HEAD, H_PER_CORE * KERNEL], F32)
            nc.sync.dma_start(out=wt, in_=w)
            wdt = wp.tile([D_HEAD, H_PER_CORE * KERNEL, D_HEAD], F16)
            nc.sync.dma_start(out=wdt, in_=wd)
            if no_dma:
                XZ = wp.tile([D_HEAD, CHUNK + PAD], F16)
                nc.vector.memset(XZ, 0.0)

            def wcol(j, k):
                hl = j // BATCH
                return wt[:, hl * KERNEL + k : hl * KERNEL + k + 1]

            # Everything is chunk-granular (2048 cols): per-chunk x tiles
            # (with a PAD-col halo), out tiles, and accumulators.  Short tile
            # live-windows give the scheduler enough SBUF slack to keep the
            # DMA train dense across repeat boundaries.
            for _r in range(repeats):
                for c in range(n_chunks):
                    lo = c * CHUNK
                    for j in streams:
                        if no_dma:
                            CX = XZ
                        else:
                            CX = xp.tile([D_HEAD, CHUNK + PAD], F16, tag="x")
                            nc.sync.dma_start(
                                out=CX, in_=x[:, j, lo : lo + CHUNK + PAD]
                            )
                        OC = op.tile([D_HEAD, CHUNK], F16, tag="o")
                        if j in pe_set:
                            hl = j // BATCH
                            # k-outer: one stationary load serves the psum
                            # unit's matmuls (accumulation groups interleave
                            # across 512-col PSUM regions, hence
                            # skip_group_check).  PSUM -> fp16 SBUF
                            # evacuation: ScalarE and VectorE each take one
                            # half of every unit concurrently, so PSUM tiles
                            # free fast enough that PE never stalls.
                            for base in range(0, CHUNK, psum_cols):
                                pt = pp.tile([D_HEAD, psum_cols], F32, tag="ps")
                                for k in range(KERNEL):
                                    for m in range(psum_cols // MM):
                                        col = base + m * MM
                                        nc.tensor.matmul(
                                            pt[:, m * MM : (m + 1) * MM],
                                            lhsT=wdt[:, hl * KERNEL + k, :],
                                            rhs=CX[:, PAD + col - k : PAD + col - k + MM],
                                            start=(k == 0),
                                            stop=(k == KERNEL - 1),
                                            skip_group_check=True,
                                        )
                                H = psum_cols // 2
                                nc.scalar.activation(
                                    OC[:, base : base + H], pt[:, 0:H],
                                    mybir.ActivationFunctionType.Copy,
                                )
                                nc.vector.tensor_copy(
                                    OC[:, base + H : base + psum_cols],
                                    pt[:, H:psum_cols],
                                )
                            if mode == "full" and half_outs:
                                # Ship each half as soon as its evac lands;
                                # ring follows the producer.
                                nc.scalar.dma_start(
                                    out=out[:, j, lo : lo + H], in_=OC[:, 0:H]
                                )
                                nc.gpsimd.dma_start(
                                    out=out[:, j, lo + H : lo + CHUNK],
                                    in_=OC[:, H:CHUNK],
                                )
                                continue
                        else:
                            A = ap.tile([D_HEAD, CHUNK], F16, tag="a")
                            # k=2 multiply into the out tile (4x mode: even
                            # shift keeps the read 4B-aligned)
                            nc.vector.tensor_scalar_mul(
                                OC, CX[:, PAD - 2 : PAD - 2 + CHUNK], wcol(j, 2)
                            )
                            nc.scalar.activation(
                                A, CX[:, PAD : PAD + CHUNK],
                                mybir.ActivationFunctionType.Copy, scale=wcol(j, 0),
                            )
                            nc.vector.scalar_tensor_tensor(
                                A, CX[:, PAD - 1 : PAD - 1 + CHUNK], wcol(j, 1),
                                A, mybir.AluOpType.mult, mybir.AluOpType.add,
                            )
                            nc.vector.scalar_tensor_tensor(
                                A, CX[:, PAD - 3 : PAD - 3 + CHUNK], wcol(j, 3),
                                A, mybir.AluOpType.mult, mybir.AluOpType.add,
                            )
                            # Pool tail add into the out tile
                            nc.gpsimd.tensor_tensor(
                                OC, A, OC, mybir.AluOpType.add,
                            )
                        if mode == "full":
                            # Out-DMA rings follow the producer: ScalarE-
                            # evacuated chunks ride the Activation HWDGE
                            # ring, VectorE-evacuated chunks ride Pool's
                            # SWDGE ring (Pool is otherwise idle) - each
                            # ring issues in its producer's completion
                            # order, avoiding FIFO head-of-line blocking.
                            if out_ring == "act":
                                eng = nc.scalar
                            elif out_ring == "sync":
                                eng = nc.sync
                            else:
                                eng = nc.scalar if j % 2 == 0 else nc.gpsimd
                            eng.dma_start(out=out[:, j, lo : lo + CHUNK], in_=OC)
    nc.compile()
    return nc


_module = None


def _get_module():
    global _module
    if _module is None:
        _module = _build_module()
    return _module


def _shard_inputs(x: np.ndarray, weight: np.ndarray, seq: int = SEQ):
    in_maps = []
    idx = np.arange(D_HEAD)
    for c in range(N_CORES):
        h0 = c * H_PER_CORE
        xs = x[:, :, h0 : h0 + H_PER_CORE, :]                # (B, T, HL, D)
        xt = xs.transpose(3, 2, 0, 1)                        # (D, HL, B, T)
        xin = np.zeros((D_HEAD, N_STREAMS, seq + PAD), dtype=np.float16)
        xin[:, :, PAD:] = xt.reshape(D_HEAD, N_STREAMS, seq).astype(np.float16)
        ws = weight[:, h0 : h0 + H_PER_CORE, :]              # (K, HL, D)
        warr = np.ascontiguousarray(ws.transpose(2, 1, 0)).reshape(
            D_HEAD, H_PER_CORE * KERNEL
        ).astype(np.float32)
        wdh = np.zeros((D_HEAD, H_PER_CORE * KERNEL, D_HEAD), dtype=np.float16)
        for hl in range(H_PER_CORE):
            for k in range(KERNEL):
                wdh[idx, hl * KERNEL + k, idx] = weight[k, h0 + hl, :].astype(np.float16)
        in_maps.append({"x": xin, "w": warr, "wd": wdh})
    return in_maps


def _unshard(results, seq: int = SEQ) -> np.ndarray:
    out = np.empty((BATCH, seq, N_HEADS, D_HEAD), dtype=np.float32)
    for c in range(N_CORES):
        h0 = c * H_PER_CORE
        o = results[c]["out"].astype(np.float32).reshape(D_HEAD, H_PER_CORE, BATCH, seq)
        out[:, :, h0 : h0 + H_PER_CORE, :] = o.transpose(2, 3, 1, 0)
    return out


def kernel(x: np.ndarray, weight: np.ndarray) -> np.ndarray:
    global last_results
    x = np.asarray(x, dtype=np.float32)
    weight = np.asarray(weight, dtype=np.float32)
    nc = _get_module()
    in_maps = _shard_inputs(x, weight)
    # The shared terminal occasionally wedges (NRT_EXEC_UNIT_UNRECOVERABLE)
    # and recovers after a pause; retry rather than fail the whole call.
    last_err = None
    for attempt in range(3):
        try:
            res = run_bass_kernel_spmd(nc, in_maps, list(range(N_CORES)))
            break
        except Exception as e:  # noqa: BLE001 - device-transient errors
            last_err = e
            time.sleep(25 * (attempt + 1))
    else:
        raise last_err
    last_results = res
    return _unshard(res.results)
